# revision 14
# baseline (speedup 1.0000x reference)
"""Trainium2 Bass kernel for triple-head Bahdanau attention (nn_Attention_48258252537865).

Reference computation (S=8192, H2=1024, A=2048, E=768):
  for each head t in {pos, cardinal, headline}:
      u_t = sentence @ W_sent_t + b_sent_t + (ctx_t @ W_ctx_t + b_ctx_t)   [1,S,A]
      e_t = tanh(u_t) @ v_t + bv_t                                          [1,S]
      w_t = softmax(mask(e_t))
  fused = (w_p + w_c + w_h) / 3
  out = fused @ sentence                                                    [1,H2]

Strategy: sequence-parallel over 8 NeuronCores; the dominant u-matmul runs in
fp8 e4m3 with DoubleRow perf mode (2 MACs/PE-cell/cycle, K=256 per matmul).
Per core (S_local = ceil(length/8) rows):
  - u tiles via fp8 DoubleRow PE matmuls (weights and sentence pre-scaled by
    2^9 / 2^5 on the host to dodge e4m3 subnormals; the 2^-14 descale rides
    the tanh activation's scale operand)
  - tanh (+ combined bias) on the scalar engine straight out of PSUM -> bf16
  - score rows for all 3 heads accumulated into one [3, S_local] PSUM tile
    via bf16 matmuls with a [128, 3] stationary holding v_t in column t;
    the additive key mask AND a host-computed fp8 error-compensation row
    enter the same accumulator via one K=3 identity matmul (f32r)
  - exp (no max shift needed: |e| <= ||v||_1 + |corr| ~ 40, host asserts the
    bound and shifts via the corr row if ever unsafe) and the local attended
    numerator N_t = sum_s exp(e_t[s]) * sentence[s,:] in bf16
The host sums the 8 cores' (Z, N) pairs exactly and returns mean_t(N_t/Z_t).

fp8 error compensation: quantizing W,x to e4m3 perturbs the logits by
  de_s ~= sum_a v_a tanh'(u_sa) (dW^T x8 + W^T dx)_sa
whose dominant (rank-1 in s) part  (x8_s . dW(v*c) + dx_s . W(v*c)) with
c_a = E_s[tanh'(u_sa)]  is computed exactly on the host (dW, dx known there)
and subtracted on-device via the corr row.  Measured: raw fp8 2.5e-2 ->
compensated 6e-3 final relative error.
"""

import numpy as np
import ml_dtypes
from contextlib import ExitStack

S = 8192
H2 = 1024
A = 2048
NCORES = 8
NEG = -1.0e30
SX = 2.0 ** 5          # sentence pre-scale (fp8)
SW = 2.0 ** 9          # weight pre-scale (fp8)
SCALE_INV = 1.0 / (SX * SW)
NP_F8 = ml_dtypes.float8_e4m3
NP_BF16 = ml_dtypes.bfloat16

_cache = {}
LAST_RESULTS = None  # BassKernelResults of the most recent device run


def _build(S_local):
    import concourse.bacc as bacc
    import concourse.tile as tile
    from concourse import mybir

    F32 = mybir.dt.float32
    F32R = mybir.dt.float32r
    F8 = mybir.dt.float8e4
    BF16 = mybir.dt.bfloat16
    TANH = mybir.ActivationFunctionType.Tanh
    EXP = mybir.ActivationFunctionType.Exp
    DR = mybir.MatmulPerfMode.DoubleRow

    KT = H2 // 128                      # 8 fp8 k-slices of the contraction
    KT2 = KT // 2                       # 4 DoubleRow K=256 tiles
    NJ = A // 128                       # a-tiles per head
    ST = S_local // 128                 # s-tiles (transpose/numerator)
    SC = [(c, min(512, S_local - c)) for c in range(0, S_local, 512)]

    nc = bacc.Bacc("TRN2", target_bir_lowering=False, debug=False,
                   num_devices=NCORES)

    # chunk-major: [p, ci*(KT*512) + k*512 + s] so each 512-col chunk is one
    # DMA with 4 KB/partition lines (fp8 per-k slices would be 512 B lines,
    # and HWDGE descriptor overhead ~60ns/line then dominates the startup)
    sentT8_d = nc.dram_tensor("sentT8", [128, KT * S_local], F8,
                              kind="ExternalInput")
    sentbf_d = nc.dram_tensor("sentbf", [128, ST * H2], BF16,
                              kind="ExternalInput")
    Wt8_d = nc.dram_tensor("Wt8", [3, NJ, 128, KT * 128], F8,
                           kind="ExternalInput")
    Vt_d = nc.dram_tensor("Vt", [128, 3 * NJ * 3], BF16, kind="ExternalInput")
    Bt_d = nc.dram_tensor("Bt", [128, 3 * NJ], F32, kind="ExternalInput")
    corr3_d = nc.dram_tensor("corr3", [3, S_local], F32R, kind="ExternalInput")
    id3r_d = nc.dram_tensor("id3r", [3, 3], F32R, kind="ExternalInput")
    id3_d = nc.dram_tensor("id3", [3, 3], F32, kind="ExternalInput")

    Ncore_d = nc.dram_tensor("Ncore", [3, H2], F32, kind="ExternalOutput")
    z3_d = nc.dram_tensor("z3", [3, 1], F32, kind="ExternalOutput")

    with tile.TileContext(nc) as tc, ExitStack() as ctx:
        const = ctx.enter_context(tc.tile_pool(name="const", bufs=1))
        wpool = ctx.enter_context(tc.tile_pool(name="w", bufs=8))
        thpool = ctx.enter_context(tc.tile_pool(name="th", bufs=4))
        # phase-1 PSUM pools (all 8 banks); closed before the epilogue pools
        # open so the banks can be reused
        ph1 = ExitStack()
        upool = ph1.enter_context(tc.tile_pool(name="u", bufs=3, space="PSUM"))
        epool = ph1.enter_context(tc.tile_pool(name="e", bufs=1, space="PSUM"))

        # ---- startup: prioritize the critical 1.6 MB.  DMA bandwidth is the
        # shared-HBM bottleneck at startup, so the bulk W stream is throttled
        # (wpool WAR: at most `bufs` tiles in flight) and the 2 MB numerator
        # operand rides the sync ring *behind* that throttle.  Rings:
        #   sync:   W(0,0..3), then lazy W stream + sentbf pieces
        #   scalar: sentence chunk k-halves [k0..3]   (first MM dependency)
        #   gpsimd: small consts, then chunk k-halves [k4..7]
        Wt_sb = {}

        def _wdma(t, j):
            w = wpool.tile([128, KT, 128], F8, tag="w")
            nc.sync.dma_start(
                w[:].rearrange("p k a -> p (k a)"), Wt8_d.ap()[t, j])
            Wt_sb[(t, j)] = w

        id3r_sb = const.tile([3, 3], F32R, tag="id3r")
        id3_sb = const.tile([3, 3], F32, tag="id3")
        Vt_sb = const.tile([128, 3 * NJ * 3], BF16, tag="vt")
        Bt_sb = const.tile([128, 3 * NJ], F32, tag="bt")
        corr3_sb = const.tile([3, S_local], F32R, tag="corr")
        KH = KT // 2
        nc.gpsimd.dma_start(id3r_sb[:], id3r_d.ap()[:])
        nc.gpsimd.dma_start(Bt_sb[:], Bt_d.ap()[:])
        nc.gpsimd.dma_start(Vt_sb[:], Vt_d.ap()[:])
        nc.gpsimd.dma_start(corr3_sb[:], corr3_d.ap()[:])
        nc.gpsimd.dma_start(id3_sb[:], id3_d.ap()[:])
        _wdma(0, 0)
        sentT_cs = []
        for ci, (c, n) in enumerate(SC):
            sct = const.tile([128, KT, n], F8, tag=f"sentT{ci}", name=f"sT{ci}")
            base = ci * KT * 512
            nc.scalar.dma_start(
                sct[:, 0:KH, :].rearrange("p k s -> p (k s)"),
                sentT8_d.ap()[:, base: base + KH * n])
            sentT_cs.append(sct)
        for ci, (c, n) in enumerate(SC):
            base = ci * KT * 512
            nc.gpsimd.dma_start(
                sentT_cs[ci][:, KH:KT, :].rearrange("p k s -> p (k s)"),
                sentT8_d.ap()[:, base + KH * n: base + KT * n])
        _wdma(0, 1)
        _wdma(0, 2)
        _wdma(0, 3)

        # numerator operand (pre-arranged on host); pieces are emitted inside
        # the loop on the sync ring so they trail the throttled W stream
        sent_sb = const.tile([128, ST * H2], BF16, tag="sent")

        # ---- score accumulator [3, S_local]: head t on partition t ----
        e3_ps = epool.tile([3, S_local], F32, tag="e")

        # ---- three heads: u (fp8 DoubleRow) -> tanh -> scores (bf16) ----
        pend = None  # tanh tile of the previous (t, j), awaiting score matmuls
        for t in range(3):
            for j in range(NJ):
                wtile = Wt_sb.pop((t, j), None)
                if wtile is None:
                    wtile = wpool.tile([128, KT, 128], F8, tag="w")
                    nc.sync.dma_start(
                        wtile[:].rearrange("p k a -> p (k a)"), Wt8_d.ap()[t, j])
                u_ps = upool.tile([128, S_local], F32, tag="u")
                for kt in range(KT2):
                    for ci, (c, n) in enumerate(SC):
                        nc.tensor.matmul(
                            u_ps[:, c:c + n],
                            wtile[:, 2 * kt:2 * kt + 2, :],
                            sentT_cs[ci][:, 2 * kt:2 * kt + 2, :],
                            start=(kt == 0), stop=(kt == KT2 - 1),
                            perf_mode=DR)
                if pend is not None:
                    pth, pt, pj = pend
                    for (c, n) in SC:
                        nc.tensor.matmul(
                            e3_ps[0:3, c:c + n],
                            Vt_sb[:, 3 * (pj * 3 + pt): 3 * (pj * 3 + pt) + 3],
                            pth[:, c:c + n],
                            start=False, stop=(pt == 2 and pj == NJ - 1))
                th = thpool.tile([128, S_local], BF16, tag="th")
                if t == 2 and j == NJ - 1:
                    # last group: per-chunk tanh so the final score matmuls
                    # start ~0.5us earlier (they head the serial epilogue)
                    for (c, n) in SC:
                        nc.scalar.activation(
                            th[:, c:c + n], u_ps[:, c:c + n], TANH,
                            bias=Bt_sb[:, j * 3 + t: j * 3 + t + 1],
                            scale=SCALE_INV)
                else:
                    nc.scalar.activation(th[:], u_ps[:], TANH,
                                         bias=Bt_sb[:, j * 3 + t: j * 3 + t + 1],
                                         scale=SCALE_INV)
                pend = (th, t, j)
                if t == 1 and j < ST:
                    # trickle the 2MB numerator operand behind the W stream
                    nc.sync.dma_start(sent_sb[:, j * H2:(j + 1) * H2],
                                      sentbf_d.ap()[:, j * H2:(j + 1) * H2])
                if t == 0 and j == 0:
                    # key mask + fp8 error-compensation rows enter the score
                    # accumulator via a K=3 identity matmul; emitted here
                    # (after the first u-group) so it doesn't head the PE
                    # queue at startup, but still precedes every score matmul
                    for (c, n) in SC:
                        nc.tensor.matmul(e3_ps[0:3, c:c + n], id3r_sb[:],
                                         corr3_sb[0:3, c:c + n],
                                         start=True, stop=False)
        pth, pt, pj = pend
        for (c, n) in SC:
            nc.tensor.matmul(e3_ps[0:3, c:c + n],
                             Vt_sb[:, 3 * (pj * 3 + pt): 3 * (pj * 3 + pt) + 3],
                             pth[:, c:c + n], start=False,
                             stop=True)

        # ---- exp + row sums straight off PSUM (no max shift needed) ----
        e3x_sb = const.tile([3, S_local], F32, tag="e3x")
        Z3 = const.tile([3, 1], F32, tag="z3")
        SCE = []
        _c = 0
        while _c < S_local:   # 256-wide first chunks so transposes start early
            _n = min(256 if _c < 512 else 512, S_local - _c)
            SCE.append((_c, _n))
            _c += _n
        zpart = const.tile([3, len(SCE)], F32, tag="zpart")
        for ci, (c, n) in enumerate(SCE):  # chunked so transposes start early
            nc.scalar.activation(e3x_sb[0:3, c:c + n], e3_ps[0:3, c:c + n], EXP,
                                 accum_out=zpart[:, ci:ci + 1])
        if len(SCE) > 1:
            nc.vector.reduce_sum(Z3[:, 0:1], zpart[:], axis=mybir.AxisListType.X)
        else:
            nc.vector.tensor_copy(Z3[:, 0:1], zpart[:, 0:1])
        nc.scalar.dma_start(z3_d.ap()[:], Z3[:])

        ph1.close()  # free u/e PSUM banks for the epilogue pools

        # ---- fused epilogue: per s-tile, transpose exp-scores to [s, 3]
        # and immediately accumulate both H2 halves of the numerator
        # N[t, :] = sum_s exp_scores[t, s] * sent[s, :] ----
        trpool = ctx.enter_context(tc.tile_pool(name="tr", bufs=3, space="PSUM"))
        npool = ctx.enter_context(tc.tile_pool(name="n", bufs=2, space="PSUM"))
        eT_sb = const.tile([128, 3 * ST], BF16, tag="eT")
        n_ps = []
        for _hi in range(H2 // 512):
            n_ps_hi = npool.tile([3, 512], F32, tag="n")
            n_ps.append(n_ps_hi)
        tr_tiles = []
        for k in range(ST):
            tr_ps = trpool.tile([128, 3], F32, tag="tr")
            nc.tensor.transpose(tr_ps[:], e3x_sb[0:3, k * 128:(k + 1) * 128],
                                id3_sb[:])
            nc.vector.tensor_copy(eT_sb[:, 3 * k:3 * k + 3], tr_ps[:])
            if k >= 1:  # numerator MMs one tile behind so PE never waits on DVE
                for hi, hc in enumerate(range(0, H2, 512)):
                    nc.tensor.matmul(n_ps[hi][0:3, :],
                                     eT_sb[:, 3 * (k - 1):3 * (k - 1) + 3],
                                     sent_sb[:, (k - 1) * H2 + hc:
                                             (k - 1) * H2 + hc + 512],
                                     start=(k == 1), stop=False)
        for hi, hc in enumerate(range(0, H2, 512)):
            nc.tensor.matmul(n_ps[hi][0:3, :],
                             eT_sb[:, 3 * (ST - 1):3 * (ST - 1) + 3],
                             sent_sb[:, (ST - 1) * H2 + hc:
                                     (ST - 1) * H2 + hc + 512],
                             start=(ST == 1), stop=True)
        n_sb = const.tile([3, H2], F32, tag="nsb")
        for hi, hc in enumerate(range(0, H2, 512)):
            # alternate engines so the two copies run in parallel
            if hi % 2 == 0:
                nc.vector.tensor_copy(n_sb[:, hc:hc + 512], n_ps[hi][:])
            else:
                nc.scalar.copy(n_sb[:, hc:hc + 512], n_ps[hi][:])
            nc.sync.dma_start(Ncore_d.ap()[:, hc:hc + 512], n_sb[:, hc:hc + 512])

    nc.compile()
    return nc


def kernel(**inputs):
    global LAST_RESULTS
    from concourse import bass_utils

    sentence = np.ascontiguousarray(
        np.asarray(inputs["sentence"], dtype=np.float32)[0])      # [S, H2]
    length = int(np.asarray(inputs["length"]).reshape(-1)[0])
    if length <= 0:
        return np.zeros((1, H2), dtype=np.float32)
    length = min(length, S)

    ctxs = [inputs["pos_embedding"], inputs["cardinal_phrase_embedding"],
            inputs["headline_embedding"]]
    tags = ["p", "c", "h"]

    # ---- host prep: quantize, fold biases, fp8 error compensation ----
    x8 = (sentence * SX).astype(NP_F8)                            # [S, H2] fp8
    x8f = x8.astype(np.float32) / SX
    dx = sentence - x8f

    bias_all = np.empty((3, A), dtype=np.float32)
    W8_all = np.empty((3, H2, A), dtype=NP_F8)
    v_all = np.empty((3, A), dtype=np.float32)
    corr_all = np.empty((3, S), dtype=np.float32)
    sub = np.arange(0, S, 16)                                     # c_a sample
    for i, tg in enumerate(tags):
        ctx = np.asarray(ctxs[i], dtype=np.float32)[0]            # [E]
        bias = (np.asarray(inputs[f"b_sent_{tg}"], dtype=np.float32)
                + ctx @ np.asarray(inputs[f"W_ctx_{tg}"], dtype=np.float32)
                + np.asarray(inputs[f"b_ctx_{tg}"], dtype=np.float32))
        W = np.asarray(inputs[f"W_sent_{tg}"], dtype=np.float32)
        v = np.asarray(inputs[f"v_{tg}"], dtype=np.float32)
        W8 = (W * SW).astype(NP_F8)
        W8f = W8.astype(np.float32) / SW
        dW = W - W8f
        u_sub = x8f[sub] @ W8f + bias
        c_a = (1.0 - np.tanh(u_sub) ** 2).mean(axis=0)            # [A]
        vc = v * c_a
        corr_all[i] = dx @ (W @ vc) + x8f @ (dW @ vc)
        bias_all[i] = bias
        W8_all[i] = W8
        v_all[i] = v

    S_local = max(128, -(-length // (NCORES * 128)) * 128)        # ceil, 128-aligned
    nc = _cache.get(S_local)
    if nc is None:
        nc = _build(S_local)
        _cache[S_local] = nc

    NJ = A // 128
    KT = H2 // 128
    # Wt8[t, j][p, kt*128 + a] = W8[t, kt*128 + p, j*128 + a]
    Wt8 = np.ascontiguousarray(
        W8_all.reshape(3, KT, 128, NJ, 128)
              .transpose(0, 3, 2, 1, 4).reshape(3, NJ, 128, KT * 128))
    # [128, (j t) * 3]: head t's v-tile in column t of its [128, 3] block
    vt_cols = v_all.T.reshape(NJ, 128, 3).transpose(1, 0, 2)      # [128, NJ, 3]
    Vt = np.zeros((128, NJ, 3, 3), dtype=np.float32)
    for t in range(3):
        Vt[:, :, t, t] = vt_cols[:, :, t]
    Vt = np.ascontiguousarray(Vt.reshape(128, 3 * NJ * 3)).astype(NP_BF16)
    Bt = np.ascontiguousarray(
        bias_all.T.reshape(NJ, 128, 3).transpose(1, 0, 2).reshape(128, 3 * NJ))
    id3 = np.eye(3, dtype=np.float32)

    # overflow guard for the shift-free exp: |e| <= ||v||_1 + max|corr|
    e_bound = max(float(np.abs(v_all[t]).sum() + np.abs(corr_all[t]).max())
                  for t in range(3))
    shift = max(0.0, e_bound - 60.0)   # exp arg stays < 60 -> < 1.2e26, Z safe
    if shift:
        corr_all -= shift              # common across cores: cancels in N/Z

    ST = S_local // 128
    in_maps = []
    for c in range(NCORES):
        s0 = c * S_local
        sl8 = x8[s0:s0 + S_local]
        slf = sentence[s0:s0 + S_local]
        if sl8.shape[0] < S_local:                                 # pad tail core
            pad = S_local - sl8.shape[0]
            sl8 = np.concatenate([sl8, np.zeros((pad, H2), NP_F8)], axis=0)
            slf = np.concatenate([slf, np.zeros((pad, H2), np.float32)], axis=0)
        # chunk-major: sentT8[p, ci*KT*512 + k*n + s'] = x8[s0+ci*512+s', k*128+p]
        slT = sl8.T.reshape(KT, 128, S_local)                      # [k, p, s]
        blocks = [
            np.ascontiguousarray(slT[:, :, cc:cc + nn].transpose(1, 0, 2)
                                 .reshape(128, KT * nn))
            for cc, nn in [(c, min(512, S_local - c))
                           for c in range(0, S_local, 512)]]
        sentT8 = np.ascontiguousarray(np.concatenate(blocks, axis=1))
        # sentbf[p, k*H2 + h] = sentence[s0 + k*128 + p, h]
        sentbf = np.ascontiguousarray(
            slf.reshape(ST, 128, H2).transpose(1, 0, 2)
               .reshape(128, ST * H2)).astype(NP_BF16)
        smask = np.where((s0 + np.arange(S_local))[None, :] < length,
                         0.0, NEG).astype(np.float32)
        corr3 = np.ascontiguousarray(
            corr_all[:, s0:s0 + S_local] if s0 + S_local <= S else
            np.pad(corr_all[:, s0:S], ((0, 0), (0, s0 + S_local - S))))
        corr3 = (corr3 + smask).astype(np.float32)
        in_maps.append(dict(
            sentT8=sentT8, sentbf=sentbf, Wt8=Wt8, Vt=Vt, Bt=Bt,
            corr3=corr3, id3r=id3, id3=id3,
        ))

    res = bass_utils.run_bass_kernel_spmd(nc, in_maps,
                                          core_ids=list(range(NCORES)))
    LAST_RESULTS = res

    # ---- exact cross-core combine (shared exp shift cancels in N/Z) ----
    Z = np.stack([res.results[c]["z3"] for c in range(NCORES)])    # [8,3,1]
    Ncore = np.stack([res.results[c]["Ncore"] for c in range(NCORES)])
    Zt = Z.astype(np.float64).sum(axis=0)[:, 0]                    # [3]
    Nt = Ncore.astype(np.float64).sum(axis=0)                      # [3,H2]
    out = (Nt / Zt[:, None]).mean(axis=0)
    return out[None, :].astype(np.float32)
